# revision 53
# baseline (speedup 1.0000x reference)
"""DilateAttention3D (3x3x3 window, dil=1) Trainium2 Bass kernel, 8-core SPMD.

Sharding: core = (b, dc) for b in {0,1}, dc in {0..3}: one batch element and a
D-chunk of 4 planes (halo 1 from zero-padded k/v) per core.

v6 (wire-minimal): the end-to-end cost is dominated by host<->device bytes,
so only raw bf16 tensors are shipped and all window gathering / layout work
runs on-chip:
 - inputs: packed q [NG,96,TB,32], k/v halo slabs [96,6,34,34], small consts.
 - Per-core tile = (dzp, y, xh): 2 D-planes x 6 heads x 16 x = 192 query
   cols (col order h,t',x); key union 4x3x18 = 216 keys in two x-halves of
   K2=108.
 - qb [128, TB, 192] rotating bufs: zeros + R-const rows persist; 6 per-head
   DMAs scatter the diagonal blocks (DMA has no partition-alignment limit).
 - k windows: Pool gathers ks_sb[4dz',3y',9x'] -> kw rows 0:96; rows 96:128
   hold the constant rank-32 exact mask factor L (exp(+60) in-window trick:
   a per-query additive constant cancels in softmax; 0/240/1 factors are
   bf16-exact).
 - v windows: Pool/ACT gather channel-major vw, then PE transposes (identity
   matmul) to key-major; DVE casts PSUM->vt bf16; vt col 96 = persistent ones
   (denominator).
 - QK/AV as before; out = 3 head-pair blocks [32,64] per tile (32-aligned
   DVE extraction from PSUM, 3x smaller than raw pa) + denominator plane,
   normalized on host.
"""
import os
import numpy as np
import ml_dtypes

BF16 = ml_dtypes.bfloat16
B, d, D, H, W = 2, 96, 16, 32, 32
NH, HD = 6, 16
DL, SLAB = 4, 6
TB = 8
NG = 16          # DMA groups of TB 2-tiles
K2 = 108         # keys per x-half (4 dz' x 3 y' x 9 x')
Q2 = 192         # query cols (6 h x 2 t' x 16 x)
NBUF = 4         # rotation depth for persistent buffer sets
BIGP = 240.0     # pre-scale mask bias; effective +60 after ACT scale=0.25

_cache = {}


def _mask_factors():
    """Exact rank-32 factorization of BIGP*mask[108, 192] per x-half.

    L[c][key, j=(t'',x'')] in {0, BIGP}; R[j, col=(h,t',x)] in {0, 1};
    L@R == BIGP*mask exactly (single matching term per entry, bf16-exact).
    """
    band = np.zeros((18, 16), np.float32)
    for x in range(16):
        band[x:x + 3, x] = 1.0
    Dm = np.zeros((4, 2), np.float32)
    for dz in range(4):
        for tp in range(2):
            Dm[dz, tp] = 1.0 if tp <= dz <= tp + 2 else 0.0
    L = np.zeros((2, K2, 32), np.float32)
    for c in range(2):
        for dz in range(4):
            for yp in range(3):
                for xp in range(9):
                    row = (dz * 3 + yp) * 9 + xp
                    L[c, row, 0:16] = BIGP * Dm[dz, 0] * band[9 * c + xp]
                    L[c, row, 16:32] = BIGP * Dm[dz, 1] * band[9 * c + xp]
    R = np.zeros((32, NH, 2, 16), np.float32)
    for tpp in range(2):
        for xpp in range(16):
            R[tpp * 16 + xpp, :, tpp, xpp] = 1.0
    return L, R.reshape(32, Q2)


def _build_nc():
    from concourse import bacc, mybir
    import concourse.tile as tile
    from contextlib import ExitStack

    f32 = mybir.dt.float32
    bf16 = mybir.dt.bfloat16
    nc = bacc.Bacc(None, target_bir_lowering=False, debug=True)

    qd_d = nc.declare_dram_parameter("qd", [NG, 96, TB, 32], bf16,
                                     isOutput=False)
    # k/v slabs ship x-padded but y-interior only; y-border rows are
    # zero-filled on-chip
    ks_d = nc.declare_dram_parameter("ks", [96, SLAB, 32, 34], bf16,
                                     isOutput=False)
    vs_d = nc.declare_dram_parameter("vs", [96, SLAB, 32, 34], bf16,
                                     isOutput=False)
    rc_d = nc.declare_dram_parameter("rc", [32, Q2], bf16, isOutput=False)
    lc_d = nc.declare_dram_parameter("lc", [32, 2, K2], bf16, isOutput=False)
    id_d = nc.declare_dram_parameter("idm", [96, 96], bf16, isOutput=False)
    out_d = nc.declare_dram_parameter("out", [NG, NH, 16, TB, 32], bf16,
                                      isOutput=True)
    den_d = nc.declare_dram_parameter("den", [NG, TB, Q2], bf16, isOutput=True)

    with ExitStack() as ctx:
        tc = ctx.enter_context(tile.TileContext(nc))
        cpool = ctx.enter_context(tc.tile_pool(name="consts", bufs=1))
        epool = ctx.enter_context(tc.tile_pool(name="amt", bufs=4))
        opool = ctx.enter_context(tc.tile_pool(name="o", bufs=3))
        pspool = ctx.enter_context(tc.tile_pool(name="ps", bufs=3, space="PSUM"))
        papool = ctx.enter_context(tc.tile_pool(name="pa", bufs=2, space="PSUM"))
        ptpool = ctx.enter_context(tc.tile_pool(name="ptr", bufs=2, space="PSUM"))

        ks_sb = cpool.tile([96, SLAB, 34, 34], bf16)
        nc.vector.memset(ks_sb[:, :, 0:1, :], 0.0)
        nc.vector.memset(ks_sb[:, :, 33:34, :], 0.0)
        nc.sync.dma_start(ks_sb[:, :, 1:33, :], ks_d[:])
        vs_sb = cpool.tile([96, SLAB, 34, 34], bf16)
        nc.vector.memset(vs_sb[:, :, 0:1, :], 0.0)
        nc.vector.memset(vs_sb[:, :, 33:34, :], 0.0)
        nc.sync.dma_start(vs_sb[:, :, 1:33, :], vs_d[:])
        id_sb = cpool.tile([96, 96], bf16)
        nc.sync.dma_start(id_sb[:], id_d[:])

        qb_bufs, kw_bufs, vw_bufs, vt_bufs = [], [], [], []
        for i in range(NBUF):
            qbb = cpool.tile([128, TB, Q2], bf16, name=f"qbbuf{i}")
            nc.vector.memset(qbb[0:96], 0.0)
            for j in range(TB):
                nc.sync.dma_start(qbb[96:128, j, :], rc_d[:])
            qb_bufs.append(qbb)
            kwb = cpool.tile([128, 2, K2], bf16, name=f"kwbuf{i}")
            nc.sync.dma_start(kwb[96:128], lc_d[:])
            kw_bufs.append(kwb)
            vwb = cpool.tile([96, 2, K2], bf16, name=f"vwbuf{i}")
            vw_bufs.append(vwb)
            vtb = cpool.tile([K2, 2, 97], bf16, name=f"vtbuf{i}")
            nc.vector.memset(vtb[:, :, 96:97], 1.0)
            vt_bufs.append(vtb)
        den_all = cpool.tile([1, NG, TB, Q2], bf16, name="den_all")

        for g_ in range(NG):
            dzp, yq = divmod(g_, 8)
            qb4 = qb_bufs[g_ % NBUF]
            for h in range(NH):
                nc.sync.dma_start(qb4[16 * h:16 * h + 16, :, 32 * h:32 * h + 32],
                                  qd_d[g_, 16 * h:16 * h + 16])
            obp = opool.tile([96, TB, Q2], bf16, tag="obp")

            for i in range(TB):
                dy, xh = divmod(i, 2)
                y = 4 * yq + dy
                x0 = 16 * xh
                t = TB * g_ + i
                kw = kw_bufs[t % NBUF]
                vw = vw_bufs[t % NBUF]
                vt = vt_bufs[t % NBUF]

                nc.gpsimd.tensor_copy(
                    kw[0:96, 0, :],
                    ks_sb[:, 2 * dzp:2 * dzp + 4, y:y + 3, x0:x0 + 9])
                nc.vector.tensor_copy(
                    kw[0:96, 1, :],
                    ks_sb[:, 2 * dzp:2 * dzp + 4, y:y + 3, x0 + 9:x0 + 18])
                nc.gpsimd.tensor_copy(
                    vw[:, 0, :],
                    vs_sb[:, 2 * dzp:2 * dzp + 4, y:y + 3, x0:x0 + 9])
                nc.scalar.copy(
                    vw[:, 1, :],
                    vs_sb[:, 2 * dzp:2 * dzp + 4, y:y + 3, x0 + 9:x0 + 18])

                ps = pspool.tile([K2, 2, Q2], f32, tag="ps")
                for c in range(2):
                    nc.tensor.matmul(
                        ps[:, c, :], lhsT=kw[:, c, :], rhs=qb4[:, i, :],
                        start=True, stop=True,
                    )
                amt = epool.tile([K2, 2, Q2], bf16, tag="amt")
                nc.scalar.activation(
                    amt[:], ps[:], mybir.ActivationFunctionType.Exp, scale=0.25
                )

                for c in range(2):
                    ptr = ptpool.tile([K2, 96], bf16, tag="ptr")
                    nc.tensor.transpose(ptr[:], vw[:, c, :], id_sb[:])
                    nc.vector.tensor_copy(vt[:, c, 0:96], ptr[:])

                pa = papool.tile([97, Q2], f32, tag="pa")
                for c in range(2):
                    nc.tensor.matmul(
                        pa[:], lhsT=vt[:, c, :], rhs=amt[:, c, :],
                        start=(c == 0), stop=(c == 1),
                    )
                nc.vector.tensor_copy(obp[:, i, :], pa[0:96, :])
                nc.vector.tensor_copy(den_all[:, g_, i, :], pa[96:97, :])
            # d2h extraction of the per-head diagonal blocks (DMA is the one
            # path allowed to address 16-row partition windows)
            for h in range(NH):
                nc.sync.dma_start(
                    out_d[g_, h],
                    obp[16 * h:16 * h + 16, :, 32 * h:32 * h + 32])
        nc.sync.dma_start(den_d[:], den_all[0:1])
    nc.compile()
    return nc


def _host_prep(q, k, v, b, dc):
    kp = np.pad(k[b], ((0, 0), (1, 1), (0, 0), (1, 1)))
    vp = np.pad(v[b], ((0, 0), (1, 1), (0, 0), (1, 1)))
    ks = kp[:, 4 * dc:4 * dc + SLAB]              # [96,6,32,34] x-padded
    vs = vp[:, 4 * dc:4 * dc + SLAB]
    qs = q[b][:, 4 * dc:4 * dc + DL]              # [96,4,32,32]

    # packed q: [g=(dzp,yq), ch, i=(dy,xh), (t',x)]
    qr = qs.reshape(96, 2, 2, 8, 4, 2, 16)        # [ch,dzp,t',yq,dy,xh,x]
    qd = qr.transpose(1, 3, 0, 4, 5, 2, 6).reshape(NG, 96, TB, 32)
    return (np.ascontiguousarray(qd.astype(BF16)),
            np.ascontiguousarray(ks.astype(BF16)),
            np.ascontiguousarray(vs.astype(BF16)))


def kernel(q, k, v):
    q = np.asarray(q, np.float32)
    k = np.asarray(k, np.float32)
    v = np.asarray(v, np.float32)

    if "nc" not in _cache:
        _cache["nc"] = _build_nc()
    nc = _cache["nc"]

    from concourse.bass_utils import run_bass_kernel_spmd

    L, R = _mask_factors()
    rc = np.ascontiguousarray(R.astype(BF16))                     # [32,192]
    lc = np.ascontiguousarray(L.transpose(2, 0, 1).astype(BF16))  # [32,2,108]
    idm = np.ascontiguousarray(np.eye(96, dtype=np.float32).astype(BF16))
    in_maps = []
    for core in range(8):
        b, dc = divmod(core, 4)
        qd, ks, vs = _host_prep(q, k, v, b, dc)
        in_maps.append({"qd": qd, "ks": ks, "vs": vs,
                        "rc": rc, "lc": lc, "idm": idm})

    res = run_bass_kernel_spmd(nc, in_maps, list(range(8)),
                               trace=bool(int(os.environ.get("KTRACE", "0"))))
    _cache["last_results"] = res

    full = np.zeros((B, D, H, W, d), np.float32)
    for core in range(8):
        b, dc = divmod(core, 4)
        ob = res.results[core]["out"].astype(np.float32)  # [NG,NH,16,TB,32]
        dn = res.results[core]["den"].astype(np.float32)  # [NG,TB,192]
        # [dzp, yq, h, c', dy, xh, tp, x]
        num = ob.reshape(2, 8, NH, 16, 4, 2, 2, 16)
        num = num.transpose(0, 6, 1, 4, 5, 7, 2, 3)  # [dzp,tp,yq,dy,xh,x,h,c']
        den = dn.reshape(2, 8, 4, 2, NH, 2, 16)      # [dzp,yq,dy,xh,h,tp,x]
        dd = den.transpose(0, 5, 1, 2, 3, 6, 4)      # [dzp,tp,yq,dy,xh,x,h]
        o = num / dd[..., None]
        full[b, 4 * dc:4 * dc + DL] = o.reshape(DL, H, W, d)
    return full


# revision 55
# speedup vs baseline: 1.2150x; 1.2150x over previous
"""DilateAttention3D (3x3x3 window, dil=1) Trainium2 Bass kernel, 8-core SPMD.

Sharding: core = (b, dc) for b in {0,1}, dc in {0..3}: one batch element and a
D-chunk of 4 planes (halo 1 from zero-padded k/v) per core.

v6 (wire-minimal): the end-to-end cost is dominated by host<->device bytes,
so only raw bf16 tensors are shipped and all window gathering / layout work
runs on-chip:
 - inputs: packed q [NG,96,TB,32], k/v halo slabs [96,6,34,34], small consts.
 - Per-core tile = (dzp, y, xh): 2 D-planes x 6 heads x 16 x = 192 query
   cols (col order h,t',x); key union 4x3x18 = 216 keys in two x-halves of
   K2=108.
 - qb [128, TB, 192] rotating bufs: zeros + R-const rows persist; 6 per-head
   DMAs scatter the diagonal blocks (DMA has no partition-alignment limit).
 - k windows: Pool gathers ks_sb[4dz',3y',9x'] -> kw rows 0:96; rows 96:128
   hold the constant rank-32 exact mask factor L (exp(+60) in-window trick:
   a per-query additive constant cancels in softmax; 0/240/1 factors are
   bf16-exact).
 - v windows: Pool/ACT gather channel-major vw, then PE transposes (identity
   matmul) to key-major; DVE casts PSUM->vt bf16; vt col 96 = persistent ones
   (denominator).
 - QK/AV as before; out = 3 head-pair blocks [32,64] per tile (32-aligned
   DVE extraction from PSUM, 3x smaller than raw pa) + denominator plane,
   normalized on host.
"""
import os
import numpy as np
import ml_dtypes

BF16 = ml_dtypes.bfloat16
B, d, D, H, W = 2, 96, 16, 32, 32
NH, HD = 6, 16
DL, SLAB = 4, 6
TB = 8
NG = 16          # DMA groups of TB 2-tiles
K2 = 108         # keys per x-half (4 dz' x 3 y' x 9 x')
Q2 = 192         # query cols (6 h x 2 t' x 16 x)
NBUF = 4         # rotation depth for persistent buffer sets
BIGP = 240.0     # pre-scale mask bias; effective +60 after ACT scale=0.25

_cache = {}


def _mask_factors():
    """Exact rank-32 factorization of BIGP*mask[108, 192] per x-half.

    L[c][key, j=(t'',x'')] in {0, BIGP}; R[j, col=(h,t',x)] in {0, 1};
    L@R == BIGP*mask exactly (single matching term per entry, bf16-exact).
    """
    band = np.zeros((18, 16), np.float32)
    for x in range(16):
        band[x:x + 3, x] = 1.0
    Dm = np.zeros((4, 2), np.float32)
    for dz in range(4):
        for tp in range(2):
            Dm[dz, tp] = 1.0 if tp <= dz <= tp + 2 else 0.0
    L = np.zeros((2, K2, 32), np.float32)
    for c in range(2):
        for dz in range(4):
            for yp in range(3):
                for xp in range(9):
                    row = (dz * 3 + yp) * 9 + xp
                    L[c, row, 0:16] = BIGP * Dm[dz, 0] * band[9 * c + xp]
                    L[c, row, 16:32] = BIGP * Dm[dz, 1] * band[9 * c + xp]
    R = np.zeros((32, NH, 2, 16), np.float32)
    for tpp in range(2):
        for xpp in range(16):
            R[tpp * 16 + xpp, :, tpp, xpp] = 1.0
    return L, R.reshape(32, Q2)


def _build_nc():
    from concourse import bacc, mybir
    import concourse.tile as tile
    from contextlib import ExitStack

    f32 = mybir.dt.float32
    bf16 = mybir.dt.bfloat16
    nc = bacc.Bacc(None, target_bir_lowering=False, debug=True)

    qd_d = nc.declare_dram_parameter("qd", [NG, 96, TB, 32], bf16,
                                     isOutput=False)
    # k/v slabs ship x-padded but y-interior only; y-border rows are
    # zero-filled on-chip
    ks_d = nc.declare_dram_parameter("ks", [96, SLAB, 32, 34], bf16,
                                     isOutput=False)
    vs_d = nc.declare_dram_parameter("vs", [96, SLAB, 32, 34], bf16,
                                     isOutput=False)
    rc_d = nc.declare_dram_parameter("rc", [32, Q2], bf16, isOutput=False)
    lc_d = nc.declare_dram_parameter("lc", [32, 2, K2], bf16, isOutput=False)
    id_d = nc.declare_dram_parameter("idm", [96, 96], bf16, isOutput=False)
    out_d = nc.declare_dram_parameter("out", [NG, NH, 16, TB, 32], bf16,
                                      isOutput=True)
    den_d = nc.declare_dram_parameter("den", [NG, TB, Q2], bf16, isOutput=True)

    with ExitStack() as ctx:
        tc = ctx.enter_context(tile.TileContext(nc))
        cpool = ctx.enter_context(tc.tile_pool(name="consts", bufs=1))
        epool = ctx.enter_context(tc.tile_pool(name="amt", bufs=4))
        opool = ctx.enter_context(tc.tile_pool(name="o", bufs=3))
        pspool = ctx.enter_context(tc.tile_pool(name="ps", bufs=3, space="PSUM"))
        papool = ctx.enter_context(tc.tile_pool(name="pa", bufs=2, space="PSUM"))
        ptpool = ctx.enter_context(tc.tile_pool(name="ptr", bufs=2, space="PSUM"))

        ks_sb = cpool.tile([96, SLAB, 34, 34], bf16)
        vs_sb = cpool.tile([96, SLAB, 34, 34], bf16)
        id_sb = cpool.tile([96, 96], bf16)
        qb_bufs, kw_bufs, vw_bufs, vt_bufs = [], [], [], []
        for i in range(NBUF):
            qb_bufs.append(cpool.tile([128, TB, Q2], bf16, name=f"qbbuf{i}"))
            kw_bufs.append(cpool.tile([128, 2, K2], bf16, name=f"kwbuf{i}"))
            vw_bufs.append(cpool.tile([96, 2, K2], bf16, name=f"vwbuf{i}"))
            vt_bufs.append(cpool.tile([K2, 2, 97], bf16, name=f"vtbuf{i}"))
        den_all = cpool.tile([1, NG, TB, Q2], bf16, name="den_all")

        def _qd_load(g):
            qbb = qb_bufs[g % NBUF]
            for h in range(NH):
                nc.sync.dma_start(qbb[16 * h:16 * h + 16, :, 32 * h:32 * h + 32],
                                  qd_d[g, 16 * h:16 * h + 16])

        # Startup FIFO order: group-0 dependencies first, late-buffer consts
        # last, so the first tiles start ~8us in instead of ~28us.
        nc.vector.memset(ks_sb[:, :, 0:1, :], 0.0)
        nc.vector.memset(ks_sb[:, :, 33:34, :], 0.0)
        nc.vector.memset(vs_sb[:, :, 0:1, :], 0.0)
        nc.vector.memset(vs_sb[:, :, 33:34, :], 0.0)
        for i in range(NBUF):
            nc.vector.memset(qb_bufs[i][0:96], 0.0)
            nc.vector.memset(vt_bufs[i][:, :, 96:97], 1.0)
        nc.sync.dma_start(ks_sb[:, :, 1:33, :], ks_d[:])
        nc.sync.dma_start(kw_bufs[0][96:128], lc_d[:])
        for j in range(TB):
            nc.sync.dma_start(qb_bufs[0][96:128, j, :], rc_d[:])
        nc.sync.dma_start(vs_sb[:, :, 1:33, :], vs_d[:])
        nc.sync.dma_start(id_sb[:], id_d[:])
        _qd_load(0)
        _qd_load(1)
        for i in range(1, NBUF):
            nc.sync.dma_start(kw_bufs[i][96:128], lc_d[:])
            for j in range(TB):
                nc.sync.dma_start(qb_bufs[i][96:128, j, :], rc_d[:])

        prev_out = None
        for g_ in range(NG):
            dzp, yq = divmod(g_, 8)
            qb4 = qb_bufs[g_ % NBUF]
            # defer the previous group's output DMAs and prefetch the
            # next-next group's q: the HWDGE FIFO is in-order, so an output
            # DMA waiting on compute would head-of-line block input loads.
            if prev_out is not None:
                po, pg = prev_out
                for h in range(NH):
                    nc.sync.dma_start(
                        out_d[pg, h],
                        po[16 * h:16 * h + 16, :, 32 * h:32 * h + 32])
            if g_ + 2 < NG:
                _qd_load(g_ + 2)
            obp = opool.tile([96, TB, Q2], bf16, tag="obp")

            for i in range(TB):
                dy, xh = divmod(i, 2)
                y = 4 * yq + dy
                x0 = 16 * xh
                t = TB * g_ + i
                kw = kw_bufs[t % NBUF]
                vw = vw_bufs[t % NBUF]
                vt = vt_bufs[t % NBUF]

                nc.gpsimd.tensor_copy(
                    kw[0:96, 0, :],
                    ks_sb[:, 2 * dzp:2 * dzp + 4, y:y + 3, x0:x0 + 9])
                nc.vector.tensor_copy(
                    kw[0:96, 1, :],
                    ks_sb[:, 2 * dzp:2 * dzp + 4, y:y + 3, x0 + 9:x0 + 18])
                nc.gpsimd.tensor_copy(
                    vw[:, 0, :],
                    vs_sb[:, 2 * dzp:2 * dzp + 4, y:y + 3, x0:x0 + 9])
                nc.scalar.copy(
                    vw[:, 1, :],
                    vs_sb[:, 2 * dzp:2 * dzp + 4, y:y + 3, x0 + 9:x0 + 18])

                ps = pspool.tile([K2, 2, Q2], f32, tag="ps")
                for c in range(2):
                    nc.tensor.matmul(
                        ps[:, c, :], lhsT=kw[:, c, :], rhs=qb4[:, i, :],
                        start=True, stop=True,
                    )
                amt = epool.tile([K2, 2, Q2], bf16, tag="amt")
                nc.scalar.activation(
                    amt[:], ps[:], mybir.ActivationFunctionType.Exp, scale=0.25
                )

                for c in range(2):
                    ptr = ptpool.tile([K2, 96], bf16, tag="ptr")
                    nc.tensor.transpose(ptr[:], vw[:, c, :], id_sb[:])
                    nc.vector.tensor_copy(vt[:, c, 0:96], ptr[:])

                pa = papool.tile([97, Q2], f32, tag="pa")
                for c in range(2):
                    nc.tensor.matmul(
                        pa[:], lhsT=vt[:, c, :], rhs=amt[:, c, :],
                        start=(c == 0), stop=(c == 1),
                    )
                nc.vector.tensor_copy(obp[:, i, :], pa[0:96, :])
                nc.vector.tensor_copy(den_all[:, g_, i, :], pa[96:97, :])
            # d2h extraction of the per-head diagonal blocks (DMA is the one
            # path allowed to address 16-row partition windows); issuance is
            # deferred to the next group iteration to avoid FIFO blocking
            prev_out = (obp, g_)
        po, pg = prev_out
        for h in range(NH):
            nc.sync.dma_start(out_d[pg, h],
                              po[16 * h:16 * h + 16, :, 32 * h:32 * h + 32])
        nc.sync.dma_start(den_d[:], den_all[0:1])
    nc.compile()
    return nc


def _host_prep(q, k, v, b, dc):
    kp = np.pad(k[b], ((0, 0), (1, 1), (0, 0), (1, 1)))
    vp = np.pad(v[b], ((0, 0), (1, 1), (0, 0), (1, 1)))
    ks = kp[:, 4 * dc:4 * dc + SLAB]              # [96,6,32,34] x-padded
    vs = vp[:, 4 * dc:4 * dc + SLAB]
    qs = q[b][:, 4 * dc:4 * dc + DL]              # [96,4,32,32]

    # packed q: [g=(dzp,yq), ch, i=(dy,xh), (t',x)]
    qr = qs.reshape(96, 2, 2, 8, 4, 2, 16)        # [ch,dzp,t',yq,dy,xh,x]
    qd = qr.transpose(1, 3, 0, 4, 5, 2, 6).reshape(NG, 96, TB, 32)
    return (np.ascontiguousarray(qd.astype(BF16)),
            np.ascontiguousarray(ks.astype(BF16)),
            np.ascontiguousarray(vs.astype(BF16)))


def kernel(q, k, v):
    q = np.asarray(q, np.float32)
    k = np.asarray(k, np.float32)
    v = np.asarray(v, np.float32)

    if "nc" not in _cache:
        _cache["nc"] = _build_nc()
    nc = _cache["nc"]

    from concourse.bass_utils import run_bass_kernel_spmd

    L, R = _mask_factors()
    rc = np.ascontiguousarray(R.astype(BF16))                     # [32,192]
    lc = np.ascontiguousarray(L.transpose(2, 0, 1).astype(BF16))  # [32,2,108]
    idm = np.ascontiguousarray(np.eye(96, dtype=np.float32).astype(BF16))
    in_maps = []
    for core in range(8):
        b, dc = divmod(core, 4)
        qd, ks, vs = _host_prep(q, k, v, b, dc)
        in_maps.append({"qd": qd, "ks": ks, "vs": vs,
                        "rc": rc, "lc": lc, "idm": idm})

    res = run_bass_kernel_spmd(nc, in_maps, list(range(8)),
                               trace=bool(int(os.environ.get("KTRACE", "0"))))
    _cache["last_results"] = res

    full = np.zeros((B, D, H, W, d), np.float32)
    for core in range(8):
        b, dc = divmod(core, 4)
        ob = res.results[core]["out"].astype(np.float32)  # [NG,NH,16,TB,32]
        dn = res.results[core]["den"].astype(np.float32)  # [NG,TB,192]
        # [dzp, yq, h, c', dy, xh, tp, x]
        num = ob.reshape(2, 8, NH, 16, 4, 2, 2, 16)
        num = num.transpose(0, 6, 1, 4, 5, 7, 2, 3)  # [dzp,tp,yq,dy,xh,x,h,c']
        den = dn.reshape(2, 8, 4, 2, NH, 2, 16)      # [dzp,yq,dy,xh,h,tp,x]
        dd = den.transpose(0, 5, 1, 2, 3, 6, 4)      # [dzp,tp,yq,dy,xh,x,h]
        o = num / dd[..., None]
        full[b, 4 * dc:4 * dc + DL] = o.reshape(DL, H, W, d)
    return full
